# revision 19
# baseline (speedup 1.0000x reference)
"""EGNN layer on 8 Trainium2 NeuronCores (Bass/Tile).

Sharding: the 1024x1024 pair grid is split by source particle i across 8
cores (128 rows each). Each core holds x,h replicated in SBUF, computes its
[128, 1024] edge block through the net_e/net_m/net_d MLPs in a
feature-on-partition layout, accumulates the scalar-m partial plus its own
x_next/h_next rows, and a 4-byte AllReduce combines m before the h update.

Layout/scheduling highlights:
  * Edge-MLP layer 1 runs as matmuls over row pairs: the h_j stream (h^T) is
    shared by all rows, so the moving operand is [h^T; dsq_hi; dsq_lo] x2
    (dsq split hi/lo bf16) against block-diagonal stationaries; the per-row
    h_i term rides in the silu bias. All weights are split hi/lo in bf16
    (two accumulating matmuls) -- plain bf16 weights bias the edge-sum m by
    ~1%.
  * Small-M matmuls (layer 2: M=16 pairs, net_m/net_d layer 2: M=9) are
    packed into persistent PSUM tiles with tile_position column tiling so
    silus run on full 128-partition tiles. Matmul operands may only start at
    partitions {0, 32, 64}; mass pairs sit at {0, 32} and odd rows are
    replicated by SBUF-to-SBUF DMA.
  * m = sum(mnet*mass): DVE product -> ones-vector matmul (partition
    reduction, PSUM-accumulated across the loop, parked in unused psB
    partitions via tile_position) -> one ACT Identity accumulate.
  * The emission is software-pipelined with a one-group skew (A/B of group
    s next to C/D of group s-1, m-accumulation a further slot behind) so
    the Tile scheduler's static per-engine orders let PE run ahead of the
    silu chain instead of ping-ponging with ACT.
"""

import contextlib

import numpy as np
import ml_dtypes

import concourse.tile as tile
import concourse.mybir as mybir
from concourse import bacc
from concourse import bass_utils

BF16 = ml_dtypes.bfloat16
F32 = mybir.dt.float32
BF = mybir.dt.bfloat16
I32 = mybir.dt.int32
AF = mybir.ActivationFunctionType
ALU = mybir.AluOpType

N, F, D, H, O = 1024, 32, 3, 64, 8
NCORES, R = 8, 128          # rows per core
NG = R // 4                 # 4-row groups per core
JH = 512                    # free-dim chunk (one PSUM bank)

_cache = {}


def _build_bass(timing=False, loop_T=None):
    nc = bacc.Bacc("TRN2", target_bir_lowering=False, debug=False,
                   num_devices=1 if timing else NCORES)

    di = {}
    for name, shape, dt in [
        ("W34dh", [68, 128], BF), ("W34dl", [68, 128], BF),
        ("A32h", [32, 64], BF), ("A32l", [32, 64], BF),
        ("We2dh", [128, 16], BF), ("We2dl", [128, 16], BF),
        ("Wmd1hr", [64, 128], BF), ("Wmd1lr", [64, 128], BF),
        ("WD9", [128, 9], BF), ("biasB", [128, 1], F32),
        ("biasC", [128, 1], F32), ("biasD", [128, 1], F32),
        ("be1c", [64, 1], F32), ("Wh1h", [32, 64], BF),
        ("whmc", [64, 1], F32), ("bh1c", [64, 1], F32),
        ("eye", [128, 128], F32), ("xT", [3, 1024], F32),
        ("hTb", [32, 1024], BF), ("xresh", [128, 8, 3], F32),
        ("xc", [128, 3], F32), ("xTc", [3, 128], F32),
        ("hTc", [32, 128], BF), ("diagjsP", [128, 32], F32),
    ]:
        di[name] = nc.dram_tensor(name, list(shape), dt,
                                  kind="ExternalInput")

    d_xout = nc.dram_tensor("xout", [128, 3], F32, kind="ExternalOutput")
    d_hT = nc.dram_tensor("hTout", [64, 128], F32, kind="ExternalOutput")
    d_m = nc.dram_tensor("mout", [1, 1], F32, kind="ExternalOutput")

    with tile.TileContext(nc) as tc:
        with (
            tc.tile_pool(name="const", bufs=1) as cp,
            tc.tile_pool(name="work", bufs=1) as wp,
            tc.tile_pool(name="ps", bufs=4, space="PSUM") as ps,
            tc.tile_pool(name="psb", bufs=1, space="PSUM") as psbp,
            tc.tile_pool(name="psd", bufs=1, space="PSUM") as psdp,
            tc.tile_pool(name="dram", bufs=1, space="DRAM") as dp,
        ):
            # ---------- constants ----------
            sb = {}
            for name, dram in di.items():
                t = cp.tile(dram.shape, dram.dtype, name="c_" + name,
                            tag="c_" + name)
                nc.sync.dma_start(t[:], dram[:])
                sb[name] = t

            io32 = cp.tile([128, 1024], I32, name="io32", tag="io32")
            nc.gpsimd.iota(io32[:], pattern=[[1, 1024]], base=0,
                           channel_multiplier=0)
            iof = cp.tile([128, 1024], F32, name="iof", tag="iof")
            nc.vector.tensor_copy(iof[:], io32[:])
            zf32 = cp.tile([128, 1024], F32, name="zf32", tag="zf32")
            nc.gpsimd.memset(zf32[:], 0.0)
            ones_bf = cp.tile([128, 1], BF, name="ones_bf", tag="ones_bf")
            nc.vector.memset(ones_bf[:], 1.0)
            ones3 = cp.tile([3, 1], F32, name="ones3", tag="ones3")
            nc.vector.memset(ones3[:], 1.0)

            massP = cp.tile([128, 1024], BF, name="massP", tag="massP")
            nc.gpsimd.memset(massP[:], 0.0)
            dcoefW = cp.tile([128, 1024], F32, name="dcoefW", tag="dcoefW")

            psB = [psbp.tile([128, JH], F32, name=f"psB{i}", tag=f"psB{i}")
                   for i in range(2)]
            psD = [psdp.tile([128, JH], F32, name=f"psD{i}", tag=f"psD{i}")
                   for i in range(2)]
            for t in psB + psD:
                nc.vector.memset(t[:], 0.0)
            pm = psB[0][64:65, :]       # m accumulator in unused psB lanes

            # ---------- precompute: dsq / dist / invd ----------
            dsq = cp.tile([128, 1024], F32, name="dsq", tag="dsq")
            xnorm = cp.tile([128, 1], F32, name="xnorm", tag="xnorm")
            sq3 = cp.tile([128, 3], F32, name="sq3", tag="sq3")
            nc.vector.tensor_mul(sq3[:], sb["xc"][:], sb["xc"][:])
            nc.scalar.activation(sq3[:], sq3[:], AF.Identity,
                                 accum_out=xnorm[:])
            xTsq = cp.tile([3, 1024], F32, name="xTsq", tag="xTsq")
            nc.vector.tensor_mul(xTsq[:], sb["xT"][:], sb["xT"][:])
            xnT = cp.tile([1, 1024], F32, name="xnT", tag="xnT")
            for h2 in range(2):
                psx = ps.tile([1, JH], F32, name="psx", tag="ps")
                nc.tensor.matmul(psx[:], ones3[:],
                                 xTsq[:, JH * h2:JH * h2 + JH],
                                 start=True, stop=True)
                nc.vector.tensor_copy(xnT[:, JH * h2:JH * h2 + JH], psx[:])
            xnTb = cp.tile([128, 1024], F32, name="xnTb", tag="xnTb")
            nc.gpsimd.partition_broadcast(xnTb[:], xnT[:])
            for h2 in range(2):
                cl = slice(JH * h2, JH * h2 + JH)
                psg = ps.tile([128, JH], F32, name="psg", tag="ps")
                nc.tensor.matmul(psg[:], sb["xTc"][:], sb["xT"][:, cl],
                                 start=True, stop=True)
                nc.vector.tensor_scalar(dsq[:, cl], psg[:], -2.0, xnorm[:],
                                        op0=ALU.mult, op1=ALU.add)
            nc.vector.tensor_add(dsq[:], dsq[:], xnTb[:])
            nc.vector.tensor_scalar(dsq[:], dsq[:], 0.0, None, op0=ALU.max)

            dist = cp.tile([128, 1024], F32, name="dist", tag="dist")
            nc.scalar.activation(dist[:], dsq[:], AF.Sqrt)
            nc.vector.tensor_scalar(dist[:], dist[:], 1.0, None, op0=ALU.add)
            invd = cp.tile([128, 1024], F32, name="invd", tag="invd")
            nc.vector.reciprocal(invd[:], dist[:])

            dhi = cp.tile([128, 1024], BF, name="dhi", tag="dhi")
            nc.vector.tensor_copy(dhi[:], dsq[:])
            dhi32 = cp.tile([128, 1024], F32, name="dhi32", tag="dhi32")
            nc.vector.tensor_copy(dhi32[:], dhi[:])
            dlo32 = cp.tile([128, 1024], F32, name="dlo32", tag="dlo32")
            nc.vector.tensor_sub(dlo32[:], dsq[:], dhi32[:])
            dlo = cp.tile([128, 1024], BF, name="dlo", tag="dlo")
            nc.vector.tensor_copy(dlo[:], dlo32[:])

            # ---------- biasA ----------
            psr = ps.tile([64, 128], F32, name="psr", tag="ps")
            nc.tensor.matmul(psr[:], sb["A32h"][:], sb["hTc"][:],
                             start=True, stop=False)
            nc.tensor.matmul(psr[:], sb["A32l"][:], sb["hTc"][:],
                             start=False, stop=True)
            rA64 = cp.tile([64, 128], F32, name="rA64", tag="rA64")
            nc.vector.tensor_scalar(rA64[:], psr[:], sb["be1c"][:], None,
                                    op0=ALU.add)
            biasA = cp.tile([128, 64], F32, name="biasA", tag="biasA")
            rAe = rA64.rearrange("p (n two) -> p two n", two=2)
            nc.sync.dma_start(biasA[0:64, :], rAe[:, 0, :])
            nc.sync.dma_start(biasA[64:128, :], rAe[:, 1, :])

            # ---------- mv tiles: [68, 4096] = 4 batches per tile ----------
            mv = []
            for k in range(2):
                t = cp.tile([68, 4096], BF, name=f"mv{k}", tag=f"mv{k}")
                for kk in range(4):
                    cl4 = slice(1024 * kk, 1024 * kk + 1024)
                    nc.sync.dma_start(t[0:32, cl4], sb["hTb"][:])
                    nc.sync.dma_start(t[34:66, cl4], sb["hTb"][:])
                mv.append(t)

            # ---------- software-pipelined main loop ----------
            st = {}

            def emit_AB(g):
                if g % 2 == 0:
                    s = g // 2
                    mvk = mv[s % 2]
                    rs = 8 * s
                    for row, dt_ in ((32, dhi), (33, dlo)):
                        nc.sync.dma_start(
                            mvk[row:row + 1, :].rearrange(
                                "p (k n) -> p k n", k=4),
                            dt_[rs:rs + 8:2, :])
                    for row, dt_ in ((66, dhi), (67, dlo)):
                        nc.sync.dma_start(
                            mvk[row:row + 1, :].rearrange(
                                "p (k n) -> p k n", k=4),
                            dt_[rs + 1:rs + 8:2, :])
                else:
                    mvk = mv[(g // 2) % 2]
                for p2 in range(2):
                    b = 2 * g + p2
                    t1 = wp.tile([128, 1024], BF, name=f"t1_{b}", tag="t1",
                                 bufs=3)
                    for h2 in range(2):
                        cl = slice(JH * h2, JH * h2 + JH)
                        psA = ps.tile([128, JH], F32, name=f"psA_{b}_{h2}",
                                      tag="ps")
                        clb = slice(1024 * (b % 4) + JH * h2,
                                    1024 * (b % 4) + JH * h2 + JH)
                        nc.tensor.matmul(psA[:], sb["W34dh"][:],
                                         mvk[:, clb], start=True, stop=False)
                        nc.tensor.matmul(psA[:], sb["W34dl"][:],
                                         mvk[:, clb], start=False, stop=True)
                        nc.scalar.activation(t1[:, cl], psA[:], AF.Silu,
                                             bias=biasA[:, b:b + 1])
                    for h2 in range(2):
                        cl = slice(JH * h2, JH * h2 + JH)
                        nc.tensor.matmul(
                            psB[h2][32 * p2:32 * p2 + 16, :], sb["We2dh"][:],
                            t1[:, cl], start=True, stop=False,
                            tile_position=(0, 32 * p2))
                        nc.tensor.matmul(
                            psB[h2][32 * p2:32 * p2 + 16, :], sb["We2dl"][:],
                            t1[:, cl], start=False, stop=True,
                            tile_position=(0, 32 * p2))
                massS = wp.tile([128, 1024], BF, name=f"massS_{g}",
                                tag="massS", bufs=3)
                for h2 in range(2):
                    cl = slice(JH * h2, JH * h2 + JH)
                    nc.scalar.activation(massS[:, cl], psB[h2][:], AF.Silu,
                                         bias=sb["biasB"][:])
                massS2 = wp.tile([64, 1024], BF, name=f"massS2_{g}",
                                 tag="massS2", bufs=3)
                nc.sync.dma_start(massS2[0:8, :], massS[8:16, :])
                nc.sync.dma_start(massS2[32:40, :], massS[40:48, :])
                st[g] = {"massS": massS, "massS2": massS2}

            def emit_CD(g):
                massS, massS2 = st[g]["massS"], st[g]["massS2"]
                for q in range(4):
                    rr = 4 * g + q
                    stile = massS if q % 2 == 0 else massS2
                    soff = 32 * (q // 2)
                    tmd = wp.tile([128, 1024], BF, name=f"tmd_{rr}",
                                  tag="tmd", bufs=3)
                    for h2 in range(2):
                        cl = slice(JH * h2, JH * h2 + JH)
                        psC = ps.tile([128, JH], F32, name=f"psC_{rr}_{h2}",
                                      tag="ps")
                        nc.tensor.matmul(
                            psC[:], sb["Wmd1hr"][soff:soff + 8, :],
                            stile[soff:soff + 8, cl],
                            start=True, stop=False)
                        nc.tensor.matmul(
                            psC[:], sb["Wmd1lr"][soff:soff + 8, :],
                            stile[soff:soff + 8, cl],
                            start=False, stop=True)
                        nc.scalar.activation(tmd[:, cl], psC[:], AF.Silu,
                                             bias=sb["biasC"][:])
                    for h2 in range(2):
                        cl = slice(JH * h2, JH * h2 + JH)
                        nc.tensor.matmul(
                            psD[h2][32 * q:32 * q + 9, :], sb["WD9"][:],
                            tmd[:, cl], start=True, stop=True,
                            tile_position=(0, 32 * q))
                mnetS = wp.tile([128, 1024], F32, name=f"mnetS_{g}",
                                tag="mnetS", bufs=2)
                for h2 in range(2):
                    cl = slice(JH * h2, JH * h2 + JH)
                    nc.scalar.activation(mnetS[:, cl], psD[h2][:], AF.Silu,
                                         bias=sb["biasD"][:])
                nc.sync.dma_start(massP[0:8, :], massS[0:8, :])
                nc.sync.dma_start(massP[32:40, :], massS[8:16, :])
                nc.sync.dma_start(massP[64:72, :], massS[32:40, :])
                nc.sync.dma_start(massP[96:104, :], massS[40:48, :])
                eq = wp.tile([128, 1024], I32, name=f"eq_{g}", tag="eq",
                             bufs=2)
                nc.vector.tensor_scalar(eq[:], iof[:],
                                        sb["diagjsP"][:, g:g + 1], None,
                                        op0=ALU.is_equal)
                nc.vector.copy_predicated(massP[:], eq[:],
                                          zf32[:, 0:512].bitcast(BF))
                prod = wp.tile([128, 1024], BF, name=f"prod_{g}", tag="prod",
                               bufs=2)
                nc.vector.tensor_mul(prod[:], mnetS[:], massP[:])
                st[g]["mnetS"] = mnetS
                st[g]["prod"] = prod

            def emit_M(g):
                mnetS, prod = st[g]["mnetS"], st[g]["prod"]
                for h2 in range(2):
                    cl = slice(JH * h2, JH * h2 + JH)
                    nc.tensor.matmul(
                        pm, ones_bf[:], prod[:, cl],
                        start=(g == 0 and h2 == 0),
                        stop=(g == NG - 1 and h2 == 1),
                        skip_group_check=True,
                        tile_position=(0, 64))
                nc.sync.dma_start(
                    dcoefW[4 * g:4 * g + 4, :],
                    mnetS.rearrange("(q k) n -> q k n", q=4)[:, 8:9, :])
                del st[g]

            loop_ctx = (tc.For_i(0, loop_T, 1) if loop_T else
                        contextlib.nullcontext())
            with loop_ctx:
                for s in range(NG + 2):
                    if s < NG:
                        emit_AB(s)
                    if 1 <= s <= NG:
                        emit_CD(s - 1)
                    if s >= 2:
                        emit_M(s - 2)

            # ---------- x update ----------
            Wx = cp.tile([128, 1024], F32, name="Wx", tag="Wx")
            nc.vector.tensor_mul(Wx[:], dcoefW[:], invd[:])
            scol = cp.tile([128, 1], F32, name="scol", tag="scol")
            junkb = cp.tile([128, 1024], BF, name="junkb", tag="junkb")
            nc.scalar.activation(junkb[:], Wx[:], AF.Identity,
                                 accum_out=scol[:])
            WxT = cp.tile([128, 8, 128], F32, name="WxT", tag="WxT")
            for k in range(8):
                psT = ps.tile([128, 128], F32, name=f"psT_{k}", tag="ps")
                nc.tensor.transpose(psT[:], Wx[:, 128 * k:128 * k + 128],
                                    sb["eye"][:])
                nc.vector.tensor_copy(WxT[:, k, :], psT[:])
            psV = ps.tile([128, 3], F32, name="psV", tag="ps")
            for k in range(8):
                nc.tensor.matmul(psV[:], WxT[:, k, :], sb["xresh"][:, k, :],
                                 start=(k == 0), stop=(k == 7),
                                 skip_group_check=True)
            tmp3 = cp.tile([128, 3], F32, name="tmp3", tag="tmp3")
            nc.vector.tensor_scalar(tmp3[:], sb["xc"][:], scol[:], None,
                                    op0=ALU.mult)
            xd = cp.tile([128, 3], F32, name="xd", tag="xd")
            nc.vector.tensor_sub(xd[:], psV[:], tmp3[:])
            xnext = cp.tile([128, 3], F32, name="xnext", tag="xnext")
            nc.vector.tensor_add(xnext[:], xd[:], sb["xc"][:])
            nc.sync.dma_start(d_xout[:], xnext[:])

            # ---------- m AllReduce + h update ----------
            junk1 = cp.tile([1, JH], F32, name="junk1", tag="junk1")
            m_loc = cp.tile([1, 1], F32, name="m_loc", tag="m_loc")
            nc.scalar.activation(junk1[:], pm, AF.Identity,
                                 accum_out=m_loc[:])
            bounce_in = dp.tile([1, 1], F32, name="bounce_in")
            bounce_out = dp.tile([1, 1], F32, name="bounce_out")
            nc.sync.dma_start(bounce_in[:], m_loc[:])
            if timing:
                nc.sync.dma_start(bounce_out[:], bounce_in[:])
            else:
                nc.gpsimd.collective_compute(
                    "AllReduce", ALU.add,
                    replica_groups=[list(range(NCORES))],
                    ins=[bounce_in.opt()], outs=[bounce_out.opt()],
                )
            m_sb = cp.tile([1, 1], F32, name="m_sb", tag="m_sb")
            nc.sync.dma_start(m_sb[:], bounce_out[:])
            nc.sync.dma_start(d_m[:], m_sb[:])
            m_bc = cp.tile([64, 1], F32, name="m_bc", tag="m_bc")
            nc.gpsimd.partition_broadcast(m_bc[:], m_sb[:])
            biasH = cp.tile([64, 1], F32, name="biasH", tag="biasH")
            nc.vector.tensor_mul(biasH[:], sb["whmc"][:], m_bc[:])
            nc.vector.tensor_add(biasH[:], biasH[:], sb["bh1c"][:])
            psH = ps.tile([64, 128], F32, name="psH", tag="ps")
            nc.tensor.matmul(psH[:], sb["Wh1h"][:], sb["hTc"][:],
                             start=True, stop=True)
            hS = cp.tile([64, 128], F32, name="hS", tag="hS")
            nc.scalar.activation(hS[:], psH[:], AF.Silu, bias=biasH[:])
            nc.sync.dma_start(d_hT[:], hS[:])

    nc.compile()
    return nc


def _prep_inputs(x, h, We1, be1, We2, be2, Wm1, bm1, Wm2, bm2,
                 Wd1, bd1, Wd2, bd2, Wh1, bh1):
    f32 = np.float32
    x = np.asarray(x, f32); h = np.asarray(h, f32)
    We1 = np.asarray(We1, f32); be1 = np.asarray(be1, f32)
    We2 = np.asarray(We2, f32); be2 = np.asarray(be2, f32)
    Wm1 = np.asarray(Wm1, f32); bm1 = np.asarray(bm1, f32)
    Wm2 = np.asarray(Wm2, f32); bm2 = np.asarray(bm2, f32)
    Wd1 = np.asarray(Wd1, f32); bd1 = np.asarray(bd1, f32)
    Wd2 = np.asarray(Wd2, f32); bd2 = np.asarray(bd2, f32)
    Wh1 = np.asarray(Wh1, f32); bh1 = np.asarray(bh1, f32)

    def hl(w):
        wh = w.astype(BF16).astype(f32)
        wl = (w - wh).astype(BF16).astype(f32)
        return wh, wl

    Bh, Bl = hl(We1[32:64])
    w1h, w1l = hl(We1[64:65])
    W34h = np.concatenate([Bh, w1h, w1h], axis=0)
    W34l = np.concatenate([Bl, w1l, np.zeros_like(w1l)], axis=0)
    W34dh = np.zeros((68, 128), f32)
    W34dh[0:34, 0:64] = W34h
    W34dh[34:68, 64:128] = W34h
    W34dl = np.zeros((68, 128), f32)
    W34dl[0:34, 0:64] = W34l
    W34dl[34:68, 64:128] = W34l
    We2h, We2l = hl(We2)
    We2dh = np.zeros((128, 16), f32)
    We2dh[0:64, 0:8] = We2h
    We2dh[64:128, 8:16] = We2h
    We2dl = np.zeros((128, 16), f32)
    We2dl[0:64, 0:8] = We2l
    We2dl[64:128, 8:16] = We2l
    Wmd1 = np.concatenate([Wm1, Wd1], axis=1)          # [8, 128]
    Wmd1h, Wmd1l = hl(Wmd1)
    Wmd1hr = np.zeros((64, 128), f32)
    Wmd1lr = np.zeros((64, 128), f32)
    for t in range(2):
        Wmd1hr[32 * t:32 * t + 8] = Wmd1h
        Wmd1lr[32 * t:32 * t + 8] = Wmd1l
    A32h, A32l = hl(We1[0:32])
    WD9 = np.zeros((128, 9), f32)
    WD9[0:64, 0:8] = Wm2
    WD9[64:128, 8:9] = Wd2
    biasB = np.zeros((128, 1), f32)
    for off in (0, 8, 32, 40):
        biasB[off:off + 8, 0] = be2
    biasC = np.zeros((128, 1), f32)
    biasC[0:64, 0] = bm1
    biasC[64:128, 0] = bd1
    biasD = np.zeros((128, 1), f32)
    for q in range(4):
        biasD[32 * q:32 * q + 8, 0] = bm2
        biasD[32 * q + 8, 0] = bd2[0]

    shared = {
        "W34dh": W34dh.astype(BF16), "W34dl": W34dl.astype(BF16),
        "A32h": A32h.astype(BF16), "A32l": A32l.astype(BF16),
        "We2dh": We2dh.astype(BF16), "We2dl": We2dl.astype(BF16),
        "Wmd1hr": Wmd1hr.astype(BF16), "Wmd1lr": Wmd1lr.astype(BF16),
        "WD9": WD9.astype(BF16),
        "biasB": biasB, "biasC": biasC, "biasD": biasD,
        "be1c": be1.reshape(64, 1),
        "Wh1h": Wh1[0:32].astype(BF16),
        "whmc": Wh1[32].reshape(64, 1),
        "bh1c": bh1.reshape(64, 1),
        "eye": np.eye(128, dtype=f32),
        "xT": np.ascontiguousarray(x.T),
        "hTb": np.ascontiguousarray(h.T).astype(BF16),
        "xresh": np.ascontiguousarray(
            x.reshape(8, 128, 3).transpose(1, 0, 2)),
    }

    in_maps = []
    for c in range(NCORES):
        rows = slice(128 * c, 128 * c + 128)
        diagjsP = np.full((128, 32), -1.0, f32)
        for g in range(32):
            for q in range(4):
                diagjsP[32 * q:32 * q + 8, g] = 128 * c + 4 * g + q
        m = dict(shared)
        m["xc"] = np.ascontiguousarray(x[rows])
        m["xTc"] = np.ascontiguousarray(x[rows].T)
        m["hTc"] = np.ascontiguousarray(h[rows].T).astype(BF16)
        m["diagjsP"] = diagjsP
        in_maps.append(m)
    return in_maps


def kernel(x, h, We1, be1, We2, be2, Wm1, bm1, Wm2, bm2,
           Wd1, bd1, Wd2, bd2, Wh1, bh1):
    if "nc" not in _cache:
        _cache["nc"] = _build_bass()
    nc = _cache["nc"]
    in_maps = _prep_inputs(x, h, We1, be1, We2, be2, Wm1, bm1, Wm2, bm2,
                           Wd1, bd1, Wd2, bd2, Wh1, bh1)
    res = bass_utils.run_bass_kernel_spmd(nc, in_maps,
                                          core_ids=list(range(NCORES)))
    x_next = np.concatenate([res.results[c]["xout"] for c in range(NCORES)],
                            axis=0)
    h_next = np.concatenate(
        [res.results[c]["hTout"].T for c in range(NCORES)], axis=0)
    return x_next, h_next
